# revision 1
# baseline (speedup 1.0000x reference)
"""Trainium2 Bass kernel for nn_AttentionMap (dense self-attention map over
feature maps): out = gamma * (v @ softmax(q^T k)^T) + x, with q/k/v 1x1-conv
projections of x.

Sharding: data-parallel over batch B=8 -> one batch element per NeuronCore.

Per-core algorithm (N = H*W = 2304, C = 256, CR = 32):
  - q = w1 @ x + b1, k = w2 @ x + b2            [32, N] bf16
  - scores are computed TRANSPOSED: sT[j, i] = sum_d k[d,j] q[d,i], so the
    softmax contraction axis j lands on PSUM partitions; exp() is applied on
    PSUM eviction with no max-subtraction (|s| < ~25, safe in fp32).  The
    K=32 contraction uses 3x tensor-engine row tiling (k/q replicated to
    partitions 32..96) so the three bank-aligned chunks stream concurrently.
  - vT_ext[j, 0:256] = gamma*(w3 @ x + b3)^T, vT_ext[j, 256] = 1.0,
    computed inside the main loop (lag-1 ahead of its consumers).
  - refineT[i, :] = sum_j E[j, i] * vT_ext[j, :] accumulated on PSUM; its
    column 256 is then exactly Z_i = sum_j exp(s[i, j]) (softmax denominator
    for free, one extra rhs column instead of a separate reduction).
  - outT[i, c] = refineT[i, c] * (1/Z_i) + xT[i, c] in one fused DVE op.
Host transposes outT back to [C, H, W].

Phase structure per core (main-loop period is ACT-bound at ~2.3us/jt):
  prologue: DMAs + PE warm-up matmuls (HAM un-throttle) + q/k (+replicas)
  A: per jt: sT (3x row-tiled) -> exp x2 -> E; vT(jt); refine i-tile 0 at
     lag-1; i-tiles 1-2 accumulate in PE slack via "bursts" through the
     shared vT PSUM slot, spilled to SBUF accumulators on the idle DVE
  B: remaining 15 refineT i-tiles, PE-bound back-to-back matmuls
"""

import json
import os
import subprocess

import numpy as np
import ml_dtypes

import concourse.bass as bass
import concourse.mybir as mybir
import concourse.tile as tile
from concourse import bass2jax as _b2j
from concourse.bass_utils import compile_bir_kernel as _orig_compile_bir_kernel
from concourse.bass_utils import run_bass_kernel_spmd

BF16 = ml_dtypes.bfloat16
F32 = mybir.dt.float32
BF = mybir.dt.bfloat16

B, C, H, W = 8, 256, 48, 48
N = H * W            # 2304
CR = C // 8          # 32
CE = C + 1           # 257: channels + ones column (softmax denominator)
NT = N // 128        # 18 tiles of 128 along both i and j
KT = C // 128        # 2 k-tiles over channels
NH = N // 2          # 1152: i-halves for PSUM double buffering

# ---------------------------------------------------------------------------
# Workaround for this walrus build's per-instruction sync-wait limit (it
# rejects any instruction carrying more than one sem wait with "Too many
# sync wait commands", CoreV3GenImpl setupSyncWait).  Tile's scheduler
# freely emits multi-wait instructions, so rewrite the BIR JSON just before
# the walrus compile: hoist all but the last wait of each instruction onto
# same-engine NoOps inserted directly before it.


def _split_multiwait_bir(bir_json: bytes) -> bytes:
    m = json.loads(bir_json)
    n = 0
    for fn in m["functions"]:
        for blk in fn["blocks"]:
            out = []
            for ins in blk["instructions"]:
                si = ins.get("sync_info")
                waits = (si or {}).get("on_wait") or []
                if len(waits) > 1:
                    for w in waits[:-1]:
                        n += 1
                        out.append({
                            "debug": ins.get("debug", 0),
                            "engine": ins["engine"],
                            "ins": [],
                            "outs": [],
                            "name": f"{ins['name']}_sw{n}",
                            "opcode": "NoOp",
                            "sync_info": {"on_wait": [w], "on_update": []},
                        })
                    si["on_wait"] = [waits[-1]]
                out.append(ins)
            blk["instructions"] = out
    return json.dumps(m).encode()


_LDW_OPT = os.environ.get("KERNEL_LDW_OPT", "0") == "1"


class _SubprocessShim:
    """Flip walrus's hardcoded --enable-ldw-opt=false (A/B via KERNEL_LDW_OPT)."""

    def __getattr__(self, a):
        return getattr(subprocess, a)

    @staticmethod
    def check_call(argv, **kw):
        if _LDW_OPT and isinstance(argv, list) and any(
            "walrus_driver" in str(a) for a in argv[:1]
        ):
            argv = ["--enable-ldw-opt=true" if a == "--enable-ldw-opt=false"
                    else a for a in argv]
        return subprocess.check_call(argv, **kw)


def _patched_compile_bir_kernel(bir_json, tmpdir, neff_name="file.neff"):
    import concourse.bass_utils as _bu
    _bu.subprocess = _SubprocessShim()
    out = _split_multiwait_bir(bytes(bir_json))
    if _LDW_OPT:
        # perturb the BIR so the compile cache doesn't serve a NEFF built
        # with the old walrus flags
        out = out.replace(b'"version"', b'"version"', 1) + b" "
        out = out.rstrip()
        m = json.loads(out)
        m["functions"][0]["blocks"][0]["instructions"].insert(0, {
            "debug": 0, "engine": "Pool", "ins": [], "outs": [],
            "name": "ldwopt_cache_buster", "opcode": "NoOp",
        })
        out = json.dumps(m).encode()
    return _orig_compile_bir_kernel(out, tmpdir, neff_name)


_b2j.compile_bir_kernel = _patched_compile_bir_kernel
# ---------------------------------------------------------------------------

WB_W = 2 * CR + CE   # packed weight columns: w1t | w2t | w3e
WB_W2 = WB_W + 2 * CR  # + REP2 = [I32 | I32] for q/k row replication


def _build_program():
    nc = bass.Bass("TRN2", target_bir_lowering=False, debug=False)

    def din(name, shape, dt):
        return nc.dram_tensor(name, shape, dt, kind="ExternalInput").ap()

    wb_d = din("wb", [128, KT, WB_W2], BF)  # w1^T | w2^T | (g*w3)^T+0col | REP2
    bsc_d = din("bsc", [CR, 2], F32)        # b1 | b2
    brow_d = din("brow", [1, CE + 128], BF)  # [g*b3, 1.0] | ones row
    x_d = din("x", [128, KT, N], BF)        # x[c, n]: c = kt*128 + p
    xt_d = din("xt", [128, NT, C], F32)     # x^T[i, c]: i = it*128 + p
    ot_d = nc.dram_tensor("ot", [128, NT, C], F32, kind="ExternalOutput").ap()

    with tile.TileContext(nc) as tc:
        with tc.tile_pool(name="const", bufs=1) as cp:
            bsc_sb = cp.tile([CR, 2], F32)
            nc.sync.dma_start(bsc_sb[:], bsc_d[:])
            brow_sb = cp.tile([1, CE + 128], BF)
            nc.sync.dma_start(brow_sb[:], brow_d[:])
            wb_sb = cp.tile([128, KT, WB_W2], BF)
            nc.sync.dma_start(wb_sb[:], wb_d[:])
            x_sb = cp.tile([128, KT, N], BF)
            for h in range(2):
                for kt in range(KT):
                    nc.sync.dma_start(x_sb[:, kt, h * NH:(h + 1) * NH],
                                      x_d[:, kt, h * NH:(h + 1) * NH])
            xt_sb = cp.tile([128, NT, C], F32)
            zb_sb = cp.tile([128, 1], F32)
            nc.vector.memset(zb_sb[:], 0.0)
            # dummy exp: pull the ACT exp table load into the DMA wait
            zs_sb = cp.tile([128, 1], F32)
            nc.scalar.activation(zs_sb[:], zb_sb[:],
                                 mybir.ActivationFunctionType.Exp,
                                 bias=zb_sb[:])

            w1t = wb_sb[:, :, 0:CR]
            w2t = wb_sb[:, :, CR:2 * CR]
            w3e = wb_sb[:, :, 2 * CR:WB_W]
            rep2 = wb_sb[0:CR, 0, WB_W:WB_W2]
            be = brow_sb[:, 0:CE]
            oner = brow_sb[:, CE:CE + 128]

            q_sb = cp.tile([3 * CR, N], BF)   # rows 0:32 data, 32:96 replicas
            k_sb = cp.tile([3 * CR, N], BF)
            vt_sb = cp.tile([128, NT, CE], BF)
            e_sb = cp.tile([128, NT, N], BF)
            sacc_sb = cp.tile([128, 2, CE], F32)  # spilled refine partials

            # ---- prologue: q, k (+ row replicas via SBUF-SBUF DMA) ------
            with tc.tile_pool(name="pqk", bufs=2, space="PSUM") as pqk:
                # dummy matmuls on the (small, early-landing) weight blob to
                # lift the PE HAM clock-gate to 8/8 while x is still in
                # flight; results are never read
                warm = pqk.tile([CR, NH], F32, tag="pqk", name="warm")
                for i in range(9):
                    nc.tensor.matmul(
                        warm[:, 0:WB_W2], wb_sb[:, 0, 0:CR],
                        wb_sb[:, i % KT, :], start=True, stop=True,
                    )
                for h in range(2):
                    for dst, wt, bi in ((q_sb, w1t, 0), (k_sb, w2t, 1)):
                        hs = slice(h * NH, (h + 1) * NH)
                        ps = pqk.tile([CR, NH], F32, tag="pqk",
                                      name=f"qk{h}{bi}")
                        for c0, cw in ((0, 512), (512, 512), (1024, 128)):
                            for kt in range(KT):
                                nc.tensor.matmul(
                                    ps[:, c0:c0 + cw],
                                    wt[:, kt, :],
                                    x_sb[:, kt, h * NH + c0:h * NH + c0 + cw],
                                    start=(kt == 0), stop=(kt == KT - 1),
                                )
                        nc.scalar.activation(
                            dst[0:CR, hs], ps[:],
                            mybir.ActivationFunctionType.Identity,
                            bias=bsc_sb[:, bi:bi + 1],
                        )
                        # replicate rows 0:32 -> 32:64, 64:96 (row groups 1/2)
                        nc.sync.dma_start(dst[CR:2 * CR, hs], dst[0:CR, hs])
                        nc.sync.dma_start(dst[2 * CR:3 * CR, hs], dst[0:CR, hs])
                nc.sync.dma_start(xt_sb[:], xt_d[:])

            # ---- main loop: sT -> exp -> E, vT, early refine(it=0) ------
            with tc.tile_pool(name="psp", bufs=2, space="PSUM") as psp, \
                 tc.tile_pool(name="pvp", bufs=1, space="PSUM") as pvp, \
                 tc.tile_pool(name="zo", bufs=3) as zo:

                def refine_mm(r_ps, it, jt):
                    nc.tensor.matmul(
                        r_ps[:],
                        e_sb[:, jt, it * 128:(it + 1) * 128],
                        vt_sb[:, jt, :],
                        start=(jt == 0), stop=(jt == NT - 1),
                    )

                def finalize(r_ps, it):
                    zinv = zo.tile([128, 1], F32, tag="zinv")
                    nc.vector.reciprocal(zinv[:], r_ps[:, C:C + 1])
                    o_sb = zo.tile([128, C], F32, tag="osb")
                    nc.vector.scalar_tensor_tensor(
                        o_sb[:], r_ps[:, 0:C], zinv[:], xt_sb[:, it, :],
                        op0=mybir.AluOpType.mult, op1=mybir.AluOpType.add,
                    )
                    nc.sync.dma_start(ot_d[:, it, :], o_sb[:])

                def vt_chain(jt):
                    ps = pvp.tile([128, CE], F32, tag="pv")
                    js = slice(jt * 128, (jt + 1) * 128)
                    nc.tensor.matmul(ps[:], x_sb[:, 0, js], w3e[:, 0, :],
                                     start=True, stop=False)
                    nc.tensor.matmul(ps[:], x_sb[:, 1, js], w3e[:, 1, :],
                                     start=False, stop=False)
                    nc.tensor.matmul(ps[:], oner[:], be[:],
                                     start=False, stop=True)
                    nc.scalar.copy(vt_sb[:, jt, :], ps[:])

                def st_chain(jt, h, packed=True):
                    js = slice(jt * 128, (jt + 1) * 128)
                    s_ps = psp.tile([128, NH], F32, tag="ps")
                    # the three bank-aligned chunks run concurrently on row
                    # groups 0/1/2 (replicas at partitions 32:96; the first
                    # iterations run unpacked while the replicas land)
                    chunks = ((0, 512, 0), (512, 512, 1), (1024, 128, 2)) \
                        if packed else ((0, 512, 0), (512, 512, 0), (1024, 128, 0))
                    for c0, cw, r in chunks:
                        nc.tensor.matmul(
                            s_ps[:, c0:c0 + cw],
                            k_sb[CR * r:CR * (r + 1), js],
                            q_sb[CR * r:CR * (r + 1),
                                 h * NH + c0:h * NH + c0 + cw],
                            start=True, stop=True,
                            tile_position=(CR * r, 0),
                        )
                    if h == 0:
                        nc.scalar.activation(
                            e_sb[:, jt, 0:NH], s_ps[:],
                            mybir.ActivationFunctionType.Exp, bias=zb_sb[:],
                        )
                    else:
                        # Schraudolph exp on the otherwise-idle DVE: the bf16
                        # bit pattern of e^s is int16(s*128/ln2 + 16250.5)
                        # (max rel err ~3.3%, diluted ~500x in the output)
                        nc.vector.tensor_scalar(
                            e_sb[:, jt, NH:2 * NH].bitcast(mybir.dt.int16),
                            s_ps[:], 184.66502, 16250.5,
                            mybir.AluOpType.mult, mybir.AluOpType.add,
                        )

                def burst(si, it, js, first):
                    # partial refine for a spilled i-tile: accumulate a few
                    # already-available j-tiles in the shared pvp slot, then
                    # fold into the SBUF accumulator
                    ps = pvp.tile([128, CE], F32, tag="pv", name=f"b{si}{js[0]}")
                    for ix, j in enumerate(js):
                        nc.tensor.matmul(
                            ps[:],
                            e_sb[:, j, it * 128:(it + 1) * 128],
                            vt_sb[:, j, :],
                            start=(ix == 0), stop=(ix == len(js) - 1),
                        )
                    if first:
                        nc.vector.tensor_copy(sacc_sb[:, si, :], ps[:])
                    else:
                        nc.vector.tensor_tensor(
                            sacc_sb[:, si, :], sacc_sb[:, si, :], ps[:],
                            mybir.AluOpType.add,
                        )

                def finalize_sacc(si, it):
                    zinv = zo.tile([128, 1], F32, tag="zinv")
                    nc.vector.reciprocal(zinv[:], sacc_sb[:, si, C:C + 1])
                    o_sb = zo.tile([128, C], F32, tag="osb")
                    nc.vector.scalar_tensor_tensor(
                        o_sb[:], sacc_sb[:, si, 0:C], zinv[:], xt_sb[:, it, :],
                        op0=mybir.AluOpType.mult, op1=mybir.AluOpType.add,
                    )
                    nc.sync.dma_start(ot_d[:, it, :], o_sb[:])

                with tc.tile_pool(name="prp", bufs=1, space="PSUM") as prp:
                    r0 = prp.tile([128, CE], F32, tag="pr")
                    for jt in range(NT):
                        st_chain(jt, 0, packed=jt >= 2)
                        st_chain(jt, 1, packed=jt >= 2)
                        vt_chain(jt)
                        if jt >= 1:
                            refine_mm(r0, 0, jt - 1)
                    refine_mm(r0, 0, NT - 1)
                    finalize(r0, 0)

            # ---- phase B: remaining refineT + finalize ------------------
            with tc.tile_pool(name="prb", bufs=2, space="PSUM") as prb, \
                 tc.tile_pool(name="zo2", bufs=3) as zo2:
                for it in range(1, NT):
                    r_ps = prb.tile([128, CE], F32, tag="prb")
                    for jt in range(NT):
                        nc.tensor.matmul(
                            r_ps[:],
                            e_sb[:, jt, it * 128:(it + 1) * 128],
                            vt_sb[:, jt, :],
                            start=(jt == 0), stop=(jt == NT - 1),
                        )
                    zinv = zo2.tile([128, 1], F32, tag="zinv2")
                    nc.vector.reciprocal(zinv[:], r_ps[:, C:C + 1])
                    o_sb = zo2.tile([128, C], F32, tag="osb2")
                    nc.vector.scalar_tensor_tensor(
                        o_sb[:], r_ps[:, 0:C], zinv[:], xt_sb[:, it, :],
                        op0=mybir.AluOpType.mult, op1=mybir.AluOpType.add,
                    )
                    nc.sync.dma_start(ot_d[:, it, :], o_sb[:])

    return nc


_NC = None


def _get_nc():
    global _NC
    if _NC is None:
        _NC = _build_program()
    return _NC


def _prep_inputs(feat_map, w1, b1, w2, b2, w3, b3, gamma):
    g = float(np.asarray(gamma))
    wb = np.zeros((C, WB_W2), np.float32)
    wb[:, 0:CR] = np.asarray(w1, np.float32).T
    wb[:, CR:2 * CR] = np.asarray(w2, np.float32).T
    wb[:, 2 * CR:2 * CR + C] = g * np.asarray(w3, np.float32).T
    wb[0:CR, WB_W:WB_W + CR] = np.eye(CR, dtype=np.float32)
    wb[0:CR, WB_W + CR:WB_W2] = np.eye(CR, dtype=np.float32)
    brow = np.zeros((1, CE + 128), np.float32)
    brow[0, 0:C] = g * np.asarray(b3, np.float32)
    brow[0, C] = 1.0
    brow[0, CE:] = 1.0
    shared = {
        "wb": np.ascontiguousarray(
            wb.reshape(KT, 128, WB_W2).transpose(1, 0, 2)
        ).astype(BF16),
        "bsc": np.stack(
            [np.asarray(b1, np.float32), np.asarray(b2, np.float32)], axis=1
        ),
        "brow": brow.astype(BF16),
    }

    fm = np.asarray(feat_map, np.float32)
    in_maps = []
    for b in range(B):
        x = fm[b].reshape(C, N)
        m = dict(shared)
        m["x"] = np.ascontiguousarray(
            x.reshape(KT, 128, N).transpose(1, 0, 2)
        ).astype(BF16)
        m["xt"] = np.ascontiguousarray(
            x.T.reshape(NT, 128, C).transpose(1, 0, 2)
        )
        in_maps.append(m)
    return in_maps


def _run(inputs, trace=False):
    nc = _get_nc()
    in_maps = _prep_inputs(**inputs)
    res = run_bass_kernel_spmd(nc, in_maps, core_ids=list(range(B)), trace=trace)
    out = np.empty((B, C, H, W), np.float32)
    for b in range(B):
        ot = res.results[b]["ot"]                      # [128, NT, C]
        o_t = ot.transpose(1, 0, 2).reshape(N, C)      # outT[i, c]
        out[b] = o_t.T.reshape(C, H, W)
    return out, res


def kernel(**inputs) -> np.ndarray:
    out, _ = _run(inputs, trace=False)
    return out

